# revision 41
# baseline (speedup 1.0000x reference)
"""Trainium2 Bass kernel for nn_AttentionBlock (B=4, S=2048, D=1024, single head).

Sharding: 8 cores = 4 batches x 2 query-halves; each core owns 1024 queries
of one batch and returns that [1024, 1024] slice of the output (transposed;
the host gather transposes it back).

Algebraic restructure (all folds host-side, x-independent):
    W2 = Wk^T @ Wq   ->  scoresT = Xk W2 Xq^T   (Q, K never materialized)
    y  = softmax(s) @ V @ Wp^T = (E @ X) @ W3^T / rowsum,  W3 = Wp @ Wv
The projection is applied AFTER the attention-weighted sum of X (z = E @ X,
then y = z @ W3^T): z is per-core-unique while VP = X @ W3^T would be
recomputed by both cores of a batch.

Precision: the scores path (G = W2' Xq^T and scoresT = Xk G) runs in
fp8 e4m3 with DoubleRow perf mode (2 K-tiles per matmul); softmax is
insensitive to ~1% score noise. W2 is pre-scaled by ALPHA=64 so its entries
are normal-range in fp8; the exp activation scale absorbs 1/ALPHA. The
output path (zT = X^T E^T, yT = W3 zT) uses fp16 operands with fp32 PSUM;
the result is returned fp16 (quantization ~3e-4, far below the fp8 noise)
and upcast on the host. Measured rel-err vs the fp32 reference: 1.65e-2.

Input DMAs ride the two hardware queues (sync + act) in exact demand
order -- per-queue FIFO is the only ordering the Tile scheduler cannot
hoist around. The act queue carries ONLY the phase-1 weight feed (it
must stay clear for the cast/exp chain: a stalled DMA issue on it blocks
phase 2); everything else queues on sync behind the phase-1 columns.
The first chunks are split small because the stream start is gated by
their arrival (~11.5us; the act hardware queue ramps ~1us later than
sync). An 8-matmul PE warmup abuts the first real matmul -- any idle gap
lets the DVFS clock decay, which is worth more than the warmup itself.

Phases (PE-major, in-order engines; PSUM = 4 double-bank tags q0..q3):
    1. G[d][128, 1024q] = sum_e W2'^T[e-pairs] @ Xq^T[e-pairs]  (fp8 DR)
       qh-outer over all 8 d-accumulators (4 tags x 2 bank-halves): the
       qh=0 pass is ee-outer (feed-paced), the qh=1 pass d-outer so each
       d-chain's cast (split DVE/ACT) pipelines behind its stop and the
       act queue is free for phase 2 at the boundary
    2. per key tile sk: scoresT = sum_d Xk^T[d-pairs] @ g8      (fp8 DR)
       qh-outer, PSUM tag sk%4 (WAR vs the draining exp lands 4 sks
       back) -> exp (ACT, scale=SCALE/ALPHA) -> expT fp16; DVE rowsum
       accumulates in fp16 (2x DVE rate, ~5e-4 rounding)
    3. rowsum via fp16 ones-column matmul (+ expT[15] as its own step so
       the boundary wait is only the last exp) -> reciprocal -> 1/r
       broadcast across partitions on the idle gpsimd engine (was two
       K=1 PE matmuls)
    4. zT[f][128d, 1024q] = sum_sk Xrow[sk,f-cols] @ expT[sk]   (fp16)
       in 2-tile f-passes with rotating PSUM tags; the softmax
       normalization (* rb) is folded into the PSUM->SBUF drain
    5. yT[e][128, 1024q] = sum_d W3^T[d][e-cols] @ zt_sb[d]     (fp16)
       groups [2,2,2,1,1]: bias adds split DVE/ACT, out-DMAs on the two
       hardware queues (gpsimd software-DGE has ~2.8us completion
       latency -- never on the kernel-end critical path). The single-tile
       groups run qh-OUTER with a per-half drain: the qh=0 half's add+DMA
       ship 1.7us before the group's last matmul, so after the final
       matmul only one [P,512] half remains (ACT add -> act-queue DMA,
       starting within ~40ns of the last stop).
"""

import numpy as np
import ml_dtypes
from contextlib import ExitStack

D = 1024
S = 2048
SQ = 1024  # queries per core
P = 128
ALPHA = 64.0  # host pre-scale on W2 so fp8 e4m3 stays in normal range
SCALE = float(1.0 / np.sqrt(np.float32(D)).astype(np.float32))
ESCALE = SCALE / ALPHA

_CACHED = {}


def _build_nc():
    import concourse.tile as tile
    from concourse import bacc, mybir

    FP = mybir.dt.float32
    F32R = mybir.dt.float32r
    F16 = mybir.dt.float16
    F8 = mybir.dt.float8e4
    Exp = mybir.ActivationFunctionType.Exp
    Copy = mybir.ActivationFunctionType.Copy
    Ident = mybir.ActivationFunctionType.Identity
    MUL = mybir.AluOpType.mult
    ADD = mybir.AluOpType.add
    DR = mybir.MatmulPerfMode.DoubleRow

    nc = bacc.Bacc("TRN2", target_bir_lowering=False)
    # x feature-major fp8 pair tiles, split by key half so every DMA line is
    # a contiguous 2KB: [p, dd, i, k] = xp[k(+SQ), 256*dd+128*i+p]
    xown8_d = nc.declare_dram_parameter("xown8", [P, 4, 2, SQ], F8, isOutput=False)
    xoth8_d = nc.declare_dram_parameter("xoth8", [P, 4, 2, SQ], F8, isOutput=False)
    # W2'^T pair tiles: [p, ee, i, d] = ALPHA*w2[d, 256*ee+128*i+p]
    w2t8_d = nc.declare_dram_parameter("w2t8", [P, 4, 2, D], F8, isOutput=False)
    # x row-major fp16: [p, sk, d] = xp[128*sk+p, d]
    xrow_d = nc.declare_dram_parameter("xrow", [P, 16, D], F16, isOutput=False)
    # W3^T fp16: [p, dt, e] = w3[e, 128*dt+p]
    w3t_d = nc.declare_dram_parameter("w3t", [P, 8, D], F16, isOutput=False)
    biasc_d = nc.declare_dram_parameter("biasc", [P, 8], FP, isOutput=False)
    yt_d = nc.declare_dram_parameter("yt", [D, SQ], F16, isOutput=True)

    ND = D // P     # 8 tiles along D
    NS = S // P     # 16 tiles along S

    with tile.TileContext(nc) as tc:
        with ExitStack() as ctx:
            pool = ctx.enter_context(tc.tile_pool(name="main", bufs=1))
            psum = ctx.enter_context(tc.tile_pool(name="psum", bufs=1, space="PSUM"))

            def ptile(shape, name, tag, dt):
                return pool.tile(list(shape), dt, name=name, tag=tag, bufs=1)

            def qbank(i, name, shape=(P, 1024)):
                # 4 PSUM tags x 2 banks each = all 8 banks
                return psum.tile(list(shape), FP, name=name, tag=f"q{i}", bufs=1)

            # ---- DMAs, split across BOTH hardware DMA queues (sync + act)
            #      so the phase-1 feed runs at 2x: W2'^T rides the act queue
            #      while the own-query columns ride the sync queue. The first
            #      matmul starts after ~256 KB per queue.
            w2t8 = ptile([P, 4, 2, D], "w2t8", "w2t8", F8)
            xown8 = ptile([P, 4, 2, SQ], "xown8", "xown8", F8)
            xoth8 = ptile([P, 4, 2, SQ], "xoth8", "xoth8", F8)
            # ---- PE p-state warmup: the PE idles while the DMA queues
            #      start; one accumulation of throwaway fp16 matmuls (memset
            #      scratch) keeps it busy so the clock is ramped when the
            #      real stream begins. 8 matmuls end right as the first
            #      phase-1 chunks land (~10.5us).
            warm_a = ptile([P, P], "warm_a", "warm_a", F16)
            warm_b = ptile([P, 512], "warm_b", "warm_b", F16)
            warm_s = ptile([1, 4], "warm_s", "warm_s", FP)
            nc.vector.memset(warm_a[:], 1.0)
            nc.vector.memset(warm_b[:], 1.0)
            pw = qbank(0, "pwarm", shape=(P, 512))
            NWARM = 8
            for j in range(NWARM):
                nc.tensor.matmul(pw[:], warm_a[:], warm_b[:],
                                 start=(j == 0), stop=(j == NWARM - 1))
            nc.vector.tensor_copy(warm_s[:], pw[0:1, 0:4])

            # Phase-1 feed: both queues' ramps run in parallel; the very
            # first matmuls need only the gp0 half of pair 0, so split those
            # chunks. The tiny constants (biasc) go on the gpsimd SOFTWARE
            # queue so they never block the hardware-queue FIFOs (in the old
            # layout a recycled-semaphore wait on the ones DMA held the sync
            # queue hostage for ~4us).
            # Feed order matches the qh-outer phase-1 consumption: all four
            # ee-chunks of the qh=0 half first, then the qh=1 halves. Early
            # demand is ~170 GB/s, under the ~358 GB/s HBM rate. The first
            # w2t8 chunk is quartered across BOTH queues (sync's hardware
            # queue ramps ~1us earlier than act's) so the d=0..3 weights
            # land by ~10.7us and the real stream starts right as the
            # trimmed 8-matmul warmup ends -- with no idle gap in between,
            # the DVFS clock stays ramped.
            nc.sync.dma_start(w2t8[:, 0, :, 0:256], w2t8_d[:, 0, :, 0:256])
            nc.scalar.dma_start(w2t8[:, 0, :, 256:768],
                                w2t8_d[:, 0, :, 256:768])
            nc.sync.dma_start(xown8[:, 0, :, 0:512], xown8_d[:, 0, :, 0:512])
            nc.scalar.dma_start(w2t8[:, 0, :, 768:D], w2t8_d[:, 0, :, 768:D])
            for ee in range(1, 4):
                nc.scalar.dma_start(w2t8[:, ee], w2t8_d[:, ee])
                nc.sync.dma_start(xown8[:, ee, :, 0:512],
                                  xown8_d[:, ee, :, 0:512])
                if ee == 2:
                    # first qh=1 chunk slips in before the last qh=0 one so
                    # the qh=1 pass never waits on it
                    nc.sync.dma_start(xown8[:, 0, :, 512:SQ],
                                      xown8_d[:, 0, :, 512:SQ])
            for ee in range(1, 4):
                nc.sync.dma_start(xown8[:, ee, :, 512:SQ],
                                  xown8_d[:, ee, :, 512:SQ])
            ones16_sb = ptile([P, 1], "ones16", "ones16", F16)
            nc.vector.memset(ones16_sb[:], 1.0)
            biasc_sb = ptile([P, 8], "biasc", "biasc", FP)
            nc.gpsimd.dma_start(biasc_sb[:], biasc_d[:, :])
            # Background inputs all ride the sync queue AFTER the phase-1
            #      feed: per-queue FIFO is the only ordering the Tile
            #      scheduler cannot hoist around, and the sync queue has no
            #      compute duties so a stalled issue never blocks the
            #      cast/exp chain (the act queue does and must stay clear).
            #      With ones/onesr gone (memset instead) the recycled-sem
            #      wait that used to hold this queue hostage for ~4us is
            #      gone too.
            # Coarse background chunks: fewer DMAs means the lazy semaphore
            # allocator recycles the END-of-kernel output-DMA sems from
            # transfers that completed long ago, so the final drain chain
            # never sits in a recycled-sem wait. Demand times (xoth8 ~30us,
            # xrow ~58us, w3t ~115us) leave huge slack over these arrivals.
            xrow = ptile([P, 16, D], "xrow", "xrow", F16)
            w3t = ptile([P, 8, D], "w3t", "w3t", F16)
            nc.sync.dma_start(xoth8[:], xoth8_d[:])
            nc.sync.dma_start(xrow[:, 0:8, :], xrow_d[:, 0:8, :])
            nc.sync.dma_start(xrow[:, 8:16, :], xrow_d[:, 8:16, :])
            nc.sync.dma_start(w3t[:], w3t_d[:])

            # ---- phase 1: G[d][128, 1024q], fp8 DoubleRow over e-pairs.
            #      qh-outer over all 8 d-accumulators (4 PSUM tags x 2
            #      bank-halves = all 8 banks): early consumption needs only
            #      the qh=0 feed chunks, at half the d-group-outer demand
            #      rate, so the HBM feed keeps up from the first chunk.
            #      The qh=0 casts overlap the qh=1 matmuls. ----
            g8 = [ptile([P, 2, SQ], f"g8_{dd}", f"g8_{dd}", F8) for dd in range(4)]
            pgs = [qbank(i, f"pg_{i}") for i in range(4)]

            def g_acc(d, qh):
                # qh=1 uses tags shifted by 2, so its WAR lands on qh=0
                # casts that completed mid-pass, not the final ones
                tag = (d // 2 + 2 * qh) % 4
                return pgs[tag][:, (d % 2) * 512:(d % 2) * 512 + 512]

            for qh in range(2):
                # both passes run ee-outer: each consumes its xown chunks
                # progressively (one per 1.73us, half the d-outer demand
                # rate, so the HBM feed keeps up for qh=1's h1 chunks too).
                # The d-chain STOPS still spread 216ns apart across the
                # final ee=3 sweep, so the casts (split DVE/ACT) pipeline
                # behind them and the act queue is free for phase-2's exp
                # chain right at the boundary.
                order = [(ee, d) for ee in range(4) for d in range(8)]
                for ee, d in order:
                    lt = w2t8[:, ee, :, d * P:(d + 1) * P]
                    nc.tensor.matmul(
                        g_acc(d, qh), lt,
                        xown8[:, ee, :, qh * 512:(qh + 1) * 512],
                        start=(ee == 0), stop=(ee == 3), perf_mode=DR)
                # split the PSUM->fp8 casts across DVE and ACT
                for d in range(8):
                    dst = g8[d // 2][:, d % 2, qh * 512:(qh + 1) * 512]
                    if d % 2 == 0:
                        nc.vector.tensor_copy(dst, g_acc(d, qh))
                    else:
                        nc.scalar.activation(dst, g_acc(d, qh), Copy)

            # ---- phase 2: scoresT (fp8 DR) -> exp -> expT fp16; rowsum acc --
            #      The accumulator is fp16 (2x DVE rate; rowsum ~2e3 with
            #      ~5e-4 relative rounding -- far below the fp8 score noise)
            #      so the phase-3 ones-matmuls run as fp16 instead of f32r.
            expT = [ptile([P, SQ], f"expT{sk}", f"expT{sk}", F16)
                    for sk in range(NS)]
            acc_sb = ptile([P, SQ], "acc_sb", "acc_sb", F16)
            for sk in range(NS):
                # rotate over all 4 PSUM tags: the WAR against the exp that
                # drains a tag lands 4 sks back (24 matmuls of slack), so a
                # late exp never stalls the PE
                psc = qbank(sk % 4, f"psc_{sk}")
                # qh-outer: the qh=0 chain only needs phase-1's qh=0 casts
                # (long done), buying the qh=1 casts extra slack; the qh=0
                # exp also fires at this chain's stop, halfway through sk.
                for qh in range(2):
                    for ee in range(4):
                        if sk < 8:
                            lt = xown8[:, ee, :, sk * P:(sk + 1) * P]
                        else:
                            lt = xoth8[:, ee, :, (sk - 8) * P:(sk - 7) * P]
                        nc.tensor.matmul(
                            psc[:, qh * 512:(qh + 1) * 512], lt,
                            g8[ee][:, :, qh * 512:(qh + 1) * 512],
                            start=(ee == 0), stop=(ee == 3), perf_mode=DR)
                for qh in range(2):
                    nc.scalar.activation(expT[sk][:, qh * 512:(qh + 1) * 512],
                                         psc[:, qh * 512:(qh + 1) * 512],
                                         Exp, scale=ESCALE)
                if sk == 0:
                    nc.vector.tensor_copy(acc_sb[:], expT[0][:])
                elif sk < NS - 1:
                    # expT[15] joins the rowsum as its own matmul step so the
                    # phase-boundary wait is only the last exp, not this chain
                    nc.vector.tensor_tensor(acc_sb[:], acc_sb[:], expT[sk][:],
                                            ADD)

            # ---- phase 3: rowsum (acc over sk 0..14, plus expT[15] as a
            #      second accumulation step) -> 1/r ----
            pc = qbank(2, "pcs", shape=(1, 1024))
            pc0, pc1 = pc[0:1, 0:512], pc[0:1, 512:1024]
            nc.tensor.matmul(pc0, ones16_sb[:], acc_sb[:, 0:512],
                             start=True, stop=False)
            nc.tensor.matmul(pc1, ones16_sb[:], acc_sb[:, 512:1024],
                             start=True, stop=False)
            nc.tensor.matmul(pc0, ones16_sb[:], expT[NS - 1][:, 0:512],
                             start=False, stop=True)
            nc.tensor.matmul(pc1, ones16_sb[:], expT[NS - 1][:, 512:1024],
                             start=False, stop=True)
            r_row = ptile([1, SQ], "r_row", "r_row", F16)
            with nc.allow_low_precision(
                    "1/rowsum in fp16 (~5e-4) -- far below the fp8 score "
                    "noise"):
                nc.vector.reciprocal(r_row[0:1, 0:512], pc0)
                nc.vector.reciprocal(r_row[0:1, 512:1024], pc1)
            # broadcast 1/r across partitions on the otherwise-idle gpsimd
            # engine (was two K=1 PE matmuls -- this frees ~2 stream slots
            # and runs entirely off the critical path)
            rb_sb = ptile([P, SQ], "rb_sb", "rb_sb", F16)
            nc.gpsimd.partition_broadcast(rb_sb[:], r_row[0:1, :])

            # ---- phase 4: zT[f][128d, 1024q] = sum_sk xrow[sk][:,f].T @
            #      expT[sk], fp16, 2-tile f-passes with rotating PSUM tags.
            #      The drain applies the softmax normalization (* rb). The r
            #      broadcast matmuls are emitted after the first group so the
            #      in-order PE never waits on the reciprocal. ----
            zt_sb = [ptile([P, SQ], f"zt{f}", f"zt{f}", F16) for f in range(ND)]
            qrot = 0
            for fg in range(4):
                otp = [qbank((qrot + i) % 4, f"ot_{fg}_{i}") for i in range(2)]
                qrot = (qrot + 2) % 4
                for sk in range(NS):
                    for i in range(2):
                        f = fg * 2 + i
                        lt = xrow[:, sk, f * P:(f + 1) * P]
                        for qh in range(2):
                            nc.tensor.matmul(
                                otp[i][:, qh * 512:(qh + 1) * 512], lt,
                                expT[sk][:, qh * 512:(qh + 1) * 512],
                                start=(sk == 0), stop=(sk == NS - 1))
                for i in range(2):
                    f = fg * 2 + i
                    nc.vector.tensor_tensor(zt_sb[f][:], otp[i][:], rb_sb[:],
                                            MUL)

            # ---- phase 5: yT[e][128, 1024q] = sum_d w3t[d][:,e].T @ zt_sb[d];
            #      bias add split across DVE (i=0) and ACT (i=1); fp16 out.
            #      Groups [2,2,2,1,1]: the final single-tile group drains
            #      with its halves split DVE||ACT and its DMAs sync||act, so
            #      the after-last-matmul chain is one [P,512] add + one DMA
            #      instead of a serialized 2-tile drain. Out-DMAs run INLINE
            #      on the issuing engine; the gpsimd software-DGE has ~2.8us
            #      completion latency, so only hardware queues at the end. ----
            ysb = [ptile([P, SQ], f"ysb_{j}", f"ysb_{j}", F16) for j in range(4)]
            egroups = [(0, 2), (2, 2), (4, 2), (6, 1), (7, 1)]
            for ebase, ecnt in egroups:
                oyp = [qbank((qrot + i) % 4, f"oy_{ebase}_{i}")
                       for i in range(ecnt)]
                qrot = (qrot + ecnt) % 4
                if ecnt == 2:
                    for d in range(ND):
                        for i in range(ecnt):
                            e = ebase + i
                            lt = w3t[:, d, e * P:(e + 1) * P]
                            for qh in range(2):
                                nc.tensor.matmul(
                                    oyp[i][:, qh * 512:(qh + 1) * 512], lt,
                                    zt_sb[d][:, qh * 512:(qh + 1) * 512],
                                    start=(d == 0), stop=(d == ND - 1))
                else:
                    # single tile, qh-OUTER: the qh=0 chain stops 8 matmuls
                    # (1.7us) before the qh=1 chain, so its bias-add and
                    # output DMA ship while qh=1 still computes -- after the
                    # last matmul only one [P,512] half remains to drain.
                    e = ebase
                    for qh in range(2):
                        for d in range(ND):
                            lt = w3t[:, d, e * P:(e + 1) * P]
                            nc.tensor.matmul(
                                oyp[0][:, qh * 512:(qh + 1) * 512], lt,
                                zt_sb[d][:, qh * 512:(qh + 1) * 512],
                                start=(d == 0), stop=(d == ND - 1))
                        cs = slice(qh * 512, (qh + 1) * 512)
                        yt = ysb[e % 4]
                        if qh == 0:
                            nc.vector.tensor_scalar_add(yt[:, cs],
                                                        oyp[0][:, cs],
                                                        biasc_sb[:, e:e + 1])
                            nc.sync.dma_start(yt_d[e * P:(e + 1) * P, cs],
                                              yt[:, cs])
                        else:
                            nc.scalar.activation(yt[:, cs], oyp[0][:, cs],
                                                 Ident,
                                                 bias=biasc_sb[:, e:e + 1])
                            nc.scalar.dma_start(yt_d[e * P:(e + 1) * P, cs],
                                                yt[:, cs])
                    continue
                for i in range(2):
                    e = ebase + i
                    yt = ysb[e % 4]
                    cs = slice(0, SQ)
                    if i == 0:
                        nc.vector.tensor_scalar_add(yt[:, cs],
                                                    oyp[i][:, cs],
                                                    biasc_sb[:, e:e + 1])
                        nc.sync.dma_start(yt_d[e * P:(e + 1) * P, cs],
                                          yt[:, cs])
                    else:
                        nc.scalar.activation(yt[:, cs], oyp[i][:, cs],
                                             Ident,
                                             bias=biasc_sb[:, e:e + 1])
                        nc.scalar.dma_start(yt_d[e * P:(e + 1) * P, cs],
                                            yt[:, cs])

    nc.compile()
    return nc


def _get_nc():
    if "nc" not in _CACHED:
        _CACHED["nc"] = _build_nc()
    return _CACHED["nc"]


def _fp8(a):
    return np.clip(a, -240.0, 240.0).astype(ml_dtypes.float8_e4m3fn)


def make_in_maps(x, w_qkv, w_proj, b_proj):
    wq = w_qkv[0:D]
    wk = w_qkv[D:2 * D]
    wv = w_qkv[2 * D:3 * D]
    w2 = wk.T @ wq                   # scoresT = Xk W2 Xq^T
    w3 = w_proj @ wv                 # y = (E X) W3^T / rowsum
    # W2'^T pair tiles [128, 4, 2, 1024]: [p, ee, i, d] = ALPHA*w2[d, .]
    w2tA = np.ascontiguousarray((ALPHA * w2).T)      # [e, d]
    w2t8 = _fp8(w2tA.reshape(4, 2, P, D).transpose(2, 0, 1, 3))
    w2t8 = np.ascontiguousarray(w2t8)
    w3t16 = np.ascontiguousarray(
        w3.T.astype(np.float16).reshape(8, P, D).transpose(1, 0, 2))
    biasc = np.ascontiguousarray(b_proj.reshape(8, P).T)
    in_maps = []
    for c in range(8):
        b, h = c // 2, c % 2
        own = x[b, h * SQ:(h + 1) * SQ]       # [1024, D] our queries
        other = x[b, (1 - h) * SQ:(2 - h) * SQ]
        xp = np.concatenate([own, other], axis=0)       # [2048, D] own-first
        xown8 = _fp8(own.T.reshape(4, 2, P, SQ).transpose(2, 0, 1, 3))
        xoth8 = _fp8(other.T.reshape(4, 2, P, SQ).transpose(2, 0, 1, 3))
        xrow16 = xp.astype(np.float16).reshape(16, P, D).transpose(1, 0, 2)
        in_maps.append({
            "xown8": np.ascontiguousarray(xown8),
            "xoth8": np.ascontiguousarray(xoth8),
            "w2t8": w2t8,
            "xrow": np.ascontiguousarray(xrow16),
            "w3t": w3t16,
            "biasc": biasc,
        })
    return in_maps


def gather_out(results):
    out = np.empty((4, S, D), dtype=np.float32)
    for c in range(8):
        b, h = c // 2, c % 2
        out[b, h * SQ:(h + 1) * SQ] = results[c]["yt"].astype(np.float32).T
    return out


def kernel(x, w_qkv, w_proj, b_proj):
    from concourse import bass_utils
    nc = _get_nc()
    in_maps = make_in_maps(np.asarray(x, dtype=np.float32),
                           np.asarray(w_qkv, dtype=np.float32),
                           np.asarray(w_proj, dtype=np.float32),
                           np.asarray(b_proj, dtype=np.float32))
    res = bass_utils.run_bass_kernel_spmd(nc, in_maps, list(range(8))).results
    return gather_out(res)



# revision 44
# speedup vs baseline: 1.0087x; 1.0087x over previous
"""Trainium2 Bass kernel for nn_AttentionBlock (B=4, S=2048, D=1024, single head).

Sharding: 8 cores = 4 batches x 2 query-halves; each core owns 1024 queries
of one batch and returns that [1024, 1024] slice of the output (transposed;
the host gather transposes it back).

Algebraic restructure (all folds host-side, x-independent):
    W2 = Wk^T @ Wq   ->  scoresT = Xk W2 Xq^T   (Q, K never materialized)
    y  = softmax(s) @ V @ Wp^T = (E @ X) @ W3^T / rowsum,  W3 = Wp @ Wv
The projection is applied AFTER the attention-weighted sum of X (z = E @ X,
then y = z @ W3^T): z is per-core-unique while VP = X @ W3^T would be
recomputed by both cores of a batch.

Precision: the scores path (G = W2' Xq^T and scoresT = Xk G) runs in
fp8 e4m3 with DoubleRow perf mode (2 K-tiles per matmul); softmax is
insensitive to ~1% score noise. W2 is pre-scaled by ALPHA=64 so its entries
are normal-range in fp8; the exp activation scale absorbs 1/ALPHA. The
output path (zT = X^T E^T, yT = W3 zT) uses fp16 operands with fp32 PSUM;
the result is returned fp16 (quantization ~3e-4, far below the fp8 noise)
and upcast on the host. Measured rel-err vs the fp32 reference: 1.65e-2.

Input DMAs ride the two hardware queues (sync + act) in exact demand
order -- per-queue FIFO is the only ordering the Tile scheduler cannot
hoist around. The act queue carries ONLY the phase-1 weight feed (it
must stay clear for the cast/exp chain: a stalled DMA issue on it blocks
phase 2); everything else queues on sync behind the phase-1 columns.
The first chunks are split small because the stream start is gated by
their arrival (~11.5us; the act hardware queue ramps ~1us later than
sync). An 8-matmul PE warmup abuts the first real matmul -- any idle gap
lets the DVFS clock decay, which is worth more than the warmup itself.

Phases (PE-major, in-order engines; PSUM = 4 double-bank tags q0..q3):
    1. G[d][128, 1024q] = sum_e W2'^T[e-pairs] @ Xq^T[e-pairs]  (fp8 DR)
       qh-outer over all 8 d-accumulators (4 tags x 2 bank-halves): the
       qh=0 pass is ee-outer (feed-paced), the qh=1 pass d-outer so each
       d-chain's cast (split DVE/ACT) pipelines behind its stop and the
       act queue is free for phase 2 at the boundary
    2. per key tile sk: scoresT = sum_d Xk^T[d-pairs] @ g8      (fp8 DR)
       qh-outer, PSUM tag sk%4 (WAR vs the draining exp lands 4 sks
       back) -> exp (ACT, scale=SCALE/ALPHA) -> expT fp16; DVE rowsum
       accumulates in fp16 (2x DVE rate, ~5e-4 rounding)
    3. rowsum via fp16 ones-column matmul (+ expT[15] as its own step so
       the boundary wait is only the last exp) -> reciprocal -> 1/r
       broadcast across partitions on the idle gpsimd engine (was two
       K=1 PE matmuls)
    4. zT[f][128d, 1024q] = sum_sk Xrow[sk,f-cols] @ expT[sk]   (fp16)
       in 2-tile f-passes with rotating PSUM tags; the softmax
       normalization (* rb) is folded into the PSUM->SBUF drain
    5. yT[e][128, 1024q] = sum_d W3^T[d][e-cols] @ zt_sb[d]     (fp16)
       groups [2,2,2,1,1]: bias adds split DVE/ACT, out-DMAs on the two
       hardware queues (gpsimd software-DGE has ~2.8us completion
       latency -- never on the kernel-end critical path). The single-tile
       groups run qh-OUTER with a per-half drain: the qh=0 half's add+DMA
       ship 1.7us before the group's last matmul, so after the final
       matmul only one [P,512] half remains (ACT add -> act-queue DMA,
       starting within ~40ns of the last stop).
"""

import numpy as np
import ml_dtypes
from contextlib import ExitStack

D = 1024
S = 2048
SQ = 1024  # queries per core
P = 128
ALPHA = 64.0  # host pre-scale on W2 so fp8 e4m3 stays in normal range
SCALE = float(1.0 / np.sqrt(np.float32(D)).astype(np.float32))
ESCALE = SCALE / ALPHA

_CACHED = {}


def _build_nc():
    import concourse.tile as tile
    from concourse import bacc, mybir

    FP = mybir.dt.float32
    F32R = mybir.dt.float32r
    F16 = mybir.dt.float16
    F8 = mybir.dt.float8e4
    Exp = mybir.ActivationFunctionType.Exp
    Copy = mybir.ActivationFunctionType.Copy
    Ident = mybir.ActivationFunctionType.Identity
    MUL = mybir.AluOpType.mult
    ADD = mybir.AluOpType.add
    DR = mybir.MatmulPerfMode.DoubleRow

    nc = bacc.Bacc("TRN2", target_bir_lowering=False)
    # x feature-major fp8 pair tiles, split by key half so every DMA line is
    # a contiguous 2KB: [p, dd, i, k] = xp[k(+SQ), 256*dd+128*i+p]
    xown8_d = nc.declare_dram_parameter("xown8", [P, 4, 2, SQ], F8, isOutput=False)
    xoth8_d = nc.declare_dram_parameter("xoth8", [P, 4, 2, SQ], F8, isOutput=False)
    # W2'^T pair tiles: [p, ee, i, d] = ALPHA*w2[d, 256*ee+128*i+p]
    w2t8_d = nc.declare_dram_parameter("w2t8", [P, 4, 2, D], F8, isOutput=False)
    # x row-major fp16: [p, sk, d] = xp[128*sk+p, d]
    xrow_d = nc.declare_dram_parameter("xrow", [P, 16, D], F16, isOutput=False)
    # W3^T fp16: [p, dt, e] = w3[e, 128*dt+p]
    w3t_d = nc.declare_dram_parameter("w3t", [P, 8, D], F16, isOutput=False)
    biasc_d = nc.declare_dram_parameter("biasc", [P, 8], FP, isOutput=False)
    yt_d = nc.declare_dram_parameter("yt", [D, SQ], F16, isOutput=True)

    ND = D // P     # 8 tiles along D
    NS = S // P     # 16 tiles along S

    with tile.TileContext(nc) as tc:
        with ExitStack() as ctx:
            pool = ctx.enter_context(tc.tile_pool(name="main", bufs=1))
            psum = ctx.enter_context(tc.tile_pool(name="psum", bufs=1, space="PSUM"))

            def ptile(shape, name, tag, dt):
                return pool.tile(list(shape), dt, name=name, tag=tag, bufs=1)

            def qbank(i, name, shape=(P, 1024)):
                # 4 PSUM tags x 2 banks each = all 8 banks
                return psum.tile(list(shape), FP, name=name, tag=f"q{i}", bufs=1)

            # ---- DMAs, split across BOTH hardware DMA queues (sync + act)
            #      so the phase-1 feed runs at 2x: W2'^T rides the act queue
            #      while the own-query columns ride the sync queue. The first
            #      matmul starts after ~256 KB per queue.
            w2t8 = ptile([P, 4, 2, D], "w2t8", "w2t8", F8)
            xown8 = ptile([P, 4, 2, SQ], "xown8", "xown8", F8)
            xoth8 = ptile([P, 4, 2, SQ], "xoth8", "xoth8", F8)
            # ---- PE p-state warmup: the PE idles while the DMA queues
            #      start; one accumulation of throwaway fp16 matmuls (memset
            #      scratch) keeps it busy so the clock is ramped when the
            #      real stream begins. 8 matmuls end right as the first
            #      phase-1 chunks land (~10.5us).
            warm_a = ptile([P, P], "warm_a", "warm_a", F16)
            warm_b = ptile([P, 512], "warm_b", "warm_b", F16)
            warm_s = ptile([1, 4], "warm_s", "warm_s", FP)
            nc.vector.memset(warm_a[:], 1.0)
            nc.vector.memset(warm_b[:], 1.0)
            pw = qbank(0, "pwarm", shape=(P, 512))
            NWARM = 8
            for j in range(NWARM):
                nc.tensor.matmul(pw[:], warm_a[:], warm_b[:],
                                 start=(j == 0), stop=(j == NWARM - 1))
            nc.vector.tensor_copy(warm_s[:], pw[0:1, 0:4])

            # Phase-1 feed: both queues' ramps run in parallel; the very
            # first matmuls need only the gp0 half of pair 0, so split those
            # chunks. The tiny constants (biasc) go on the gpsimd SOFTWARE
            # queue so they never block the hardware-queue FIFOs (in the old
            # layout a recycled-semaphore wait on the ones DMA held the sync
            # queue hostage for ~4us).
            # Feed order matches the qh-outer phase-1 consumption: all four
            # ee-chunks of the qh=0 half first, then the qh=1 halves. Early
            # demand is ~170 GB/s, under the ~358 GB/s HBM rate. The first
            # w2t8 chunk is quartered across BOTH queues (sync's hardware
            # queue ramps ~1us earlier than act's) so the d=0..3 weights
            # land by ~10.7us and the real stream starts right as the
            # trimmed 8-matmul warmup ends -- with no idle gap in between,
            # the DVFS clock stays ramped.
            nc.sync.dma_start(w2t8[:, 0, :, 0:256], w2t8_d[:, 0, :, 0:256])
            nc.scalar.dma_start(w2t8[:, 0, :, 256:768],
                                w2t8_d[:, 0, :, 256:768])
            nc.sync.dma_start(xown8[:, 0, :, 0:512], xown8_d[:, 0, :, 0:512])
            nc.scalar.dma_start(w2t8[:, 0, :, 768:D], w2t8_d[:, 0, :, 768:D])
            for ee in range(1, 4):
                nc.scalar.dma_start(w2t8[:, ee], w2t8_d[:, ee])
                nc.sync.dma_start(xown8[:, ee, :, 0:512],
                                  xown8_d[:, ee, :, 0:512])
                if ee == 2:
                    # first qh=1 chunk slips in before the last qh=0 one so
                    # the qh=1 pass never waits on it
                    nc.sync.dma_start(xown8[:, 0, :, 512:SQ],
                                      xown8_d[:, 0, :, 512:SQ])
            for ee in range(1, 4):
                nc.sync.dma_start(xown8[:, ee, :, 512:SQ],
                                  xown8_d[:, ee, :, 512:SQ])
            ones16_sb = ptile([P, 1], "ones16", "ones16", F16)
            nc.vector.memset(ones16_sb[:], 1.0)
            biasc_sb = ptile([P, 8], "biasc", "biasc", FP)
            nc.gpsimd.dma_start(biasc_sb[:], biasc_d[:, :])
            # Background inputs all ride the sync queue AFTER the phase-1
            #      feed: per-queue FIFO is the only ordering the Tile
            #      scheduler cannot hoist around, and the sync queue has no
            #      compute duties so a stalled issue never blocks the
            #      cast/exp chain (the act queue does and must stay clear).
            #      With ones/onesr gone (memset instead) the recycled-sem
            #      wait that used to hold this queue hostage for ~4us is
            #      gone too.
            # Coarse background chunks: fewer DMAs means the lazy semaphore
            # allocator recycles the END-of-kernel output-DMA sems from
            # transfers that completed long ago, so the final drain chain
            # never sits in a recycled-sem wait. Demand times (xoth8 ~30us,
            # xrow ~58us, w3t ~115us) leave huge slack over these arrivals.
            xrow = ptile([P, 16, D], "xrow", "xrow", F16)
            w3t = ptile([P, 8, D], "w3t", "w3t", F16)
            nc.sync.dma_start(xoth8[:], xoth8_d[:])
            nc.sync.dma_start(xrow[:, 0:8, :], xrow_d[:, 0:8, :])
            nc.sync.dma_start(xrow[:, 8:16, :], xrow_d[:, 8:16, :])
            nc.sync.dma_start(w3t[:], w3t_d[:])

            # ---- phase 1: G[d][128, 1024q], fp8 DoubleRow over e-pairs.
            #      qh-outer over all 8 d-accumulators (4 PSUM tags x 2
            #      bank-halves = all 8 banks): early consumption needs only
            #      the qh=0 feed chunks, at half the d-group-outer demand
            #      rate, so the HBM feed keeps up from the first chunk.
            #      The qh=0 casts overlap the qh=1 matmuls. ----
            g8 = [ptile([P, 2, SQ], f"g8_{dd}", f"g8_{dd}", F8) for dd in range(4)]
            pgs = [qbank(i, f"pg_{i}") for i in range(4)]

            def g_acc(d, qh):
                # qh=1 uses tags shifted by 2, so its WAR lands on qh=0
                # casts that completed mid-pass, not the final ones
                tag = (d // 2 + 2 * qh) % 4
                return pgs[tag][:, (d % 2) * 512:(d % 2) * 512 + 512]

            for qh in range(2):
                # qh=0 runs ee-outer (feed-paced: each ee chunk is consumed
                # once, at half the old demand rate). qh=1 runs d-outer so
                # each d-chain STOPS early and its cast pipelines behind it
                # -- by phase-1 end only the d=7 cast remains, so the act
                # queue is free for phase-2's exp chain almost immediately.
                if qh == 0:
                    order = [(ee, d) for ee in range(4) for d in range(8)]
                else:
                    order = [(ee, d) for d in range(8) for ee in range(4)]
                for ee, d in order:
                    lt = w2t8[:, ee, :, d * P:(d + 1) * P]
                    nc.tensor.matmul(
                        g_acc(d, qh), lt,
                        xown8[:, ee, :, qh * 512:(qh + 1) * 512],
                        start=(ee == 0), stop=(ee == 3), perf_mode=DR)
                # split the PSUM->fp8 casts across DVE and ACT
                for d in range(8):
                    dst = g8[d // 2][:, d % 2, qh * 512:(qh + 1) * 512]
                    if d % 2 == 0:
                        nc.vector.tensor_copy(dst, g_acc(d, qh))
                    else:
                        nc.scalar.activation(dst, g_acc(d, qh), Copy)

            # ---- phase 2: scoresT (fp8 DR) -> exp -> expT fp16; rowsum acc --
            #      The accumulator is fp16 (2x DVE rate; rowsum ~2e3 with
            #      ~5e-4 relative rounding -- far below the fp8 score noise)
            #      so the phase-3 ones-matmuls run as fp16 instead of f32r.
            expT = [ptile([P, SQ], f"expT{sk}", f"expT{sk}", F16)
                    for sk in range(NS)]
            acc_sb = ptile([P, SQ], "acc_sb", "acc_sb", F16)
            for sk in range(NS):
                # rotate over all 4 PSUM tags: the WAR against the exp that
                # drains a tag lands 4 sks back (24 matmuls of slack), so a
                # late exp never stalls the PE
                psc = qbank(sk % 4, f"psc_{sk}")
                # qh-outer: the qh=0 chain only needs phase-1's qh=0 casts
                # (long done), buying the qh=1 casts extra slack; the qh=0
                # exp also fires at this chain's stop, halfway through sk.
                for qh in range(2):
                    for ee in range(4):
                        if sk < 8:
                            lt = xown8[:, ee, :, sk * P:(sk + 1) * P]
                        else:
                            lt = xoth8[:, ee, :, (sk - 8) * P:(sk - 7) * P]
                        nc.tensor.matmul(
                            psc[:, qh * 512:(qh + 1) * 512], lt,
                            g8[ee][:, :, qh * 512:(qh + 1) * 512],
                            start=(ee == 0), stop=(ee == 3), perf_mode=DR)
                for qh in range(2):
                    nc.scalar.activation(expT[sk][:, qh * 512:(qh + 1) * 512],
                                         psc[:, qh * 512:(qh + 1) * 512],
                                         Exp, scale=ESCALE)
                if sk == 0:
                    nc.vector.tensor_copy(acc_sb[:], expT[0][:])
                elif sk < NS - 1:
                    # expT[15] joins the rowsum as its own matmul step so the
                    # phase-boundary wait is only the last exp, not this chain
                    nc.vector.tensor_tensor(acc_sb[:], acc_sb[:], expT[sk][:],
                                            ADD)

            # ---- phase 3: rowsum (acc over sk 0..14, plus expT[15] as a
            #      second accumulation step) -> 1/r ----
            pc = qbank(2, "pcs", shape=(1, 1024))
            pc0, pc1 = pc[0:1, 0:512], pc[0:1, 512:1024]
            nc.tensor.matmul(pc0, ones16_sb[:], acc_sb[:, 0:512],
                             start=True, stop=False)
            nc.tensor.matmul(pc1, ones16_sb[:], acc_sb[:, 512:1024],
                             start=True, stop=False)
            nc.tensor.matmul(pc0, ones16_sb[:], expT[NS - 1][:, 0:512],
                             start=False, stop=True)
            nc.tensor.matmul(pc1, ones16_sb[:], expT[NS - 1][:, 512:1024],
                             start=False, stop=True)
            r_row = ptile([1, SQ], "r_row", "r_row", F16)
            with nc.allow_low_precision(
                    "1/rowsum in fp16 (~5e-4) -- far below the fp8 score "
                    "noise"):
                nc.vector.reciprocal(r_row[0:1, 0:512], pc0)
                nc.vector.reciprocal(r_row[0:1, 512:1024], pc1)
            # broadcast 1/r across partitions on the otherwise-idle gpsimd
            # engine (was two K=1 PE matmuls -- this frees ~2 stream slots
            # and runs entirely off the critical path)
            rb_sb = ptile([P, SQ], "rb_sb", "rb_sb", F16)
            nc.gpsimd.partition_broadcast(rb_sb[:], r_row[0:1, :])

            # ---- phase 4: zT[f][128d, 1024q] = sum_sk xrow[sk][:,f].T @
            #      expT[sk], fp16, 2-tile f-passes with rotating PSUM tags.
            #      The drain applies the softmax normalization (* rb). The r
            #      broadcast matmuls are emitted after the first group so the
            #      in-order PE never waits on the reciprocal. ----
            zt_sb = [ptile([P, SQ], f"zt{f}", f"zt{f}", F16) for f in range(ND)]
            qrot = 0
            for fg in range(4):
                otp = [qbank((qrot + i) % 4, f"ot_{fg}_{i}") for i in range(2)]
                qrot = (qrot + 2) % 4
                for sk in range(NS):
                    for i in range(2):
                        f = fg * 2 + i
                        lt = xrow[:, sk, f * P:(f + 1) * P]
                        for qh in range(2):
                            nc.tensor.matmul(
                                otp[i][:, qh * 512:(qh + 1) * 512], lt,
                                expT[sk][:, qh * 512:(qh + 1) * 512],
                                start=(sk == 0), stop=(sk == NS - 1))
                for i in range(2):
                    f = fg * 2 + i
                    nc.vector.tensor_tensor(zt_sb[f][:], otp[i][:], rb_sb[:],
                                            MUL)

            # ---- phase 5: yT[e][128, 1024q] = sum_d w3t[d][:,e].T @ zt_sb[d];
            #      bias add split across DVE (i=0) and ACT (i=1); fp16 out.
            #      Groups [2,2,2,1,1]: the final single-tile group drains
            #      with its halves split DVE||ACT and its DMAs sync||act, so
            #      the after-last-matmul chain is one [P,512] add + one DMA
            #      instead of a serialized 2-tile drain. Out-DMAs run INLINE
            #      on the issuing engine; the gpsimd software-DGE has ~2.8us
            #      completion latency, so only hardware queues at the end. ----
            ysb = [ptile([P, SQ], f"ysb_{j}", f"ysb_{j}", F16) for j in range(4)]
            egroups = [(0, 2), (2, 2), (4, 2), (6, -2)]
            for ebase, ecnt in egroups:
                oyp = [qbank((qrot + i) % 4, f"oy_{ebase}_{i}")
                       for i in range(abs(ecnt))]
                qrot = (qrot + abs(ecnt)) % 4
                if ecnt == 2:
                    for d in range(ND):
                        for i in range(ecnt):
                            e = ebase + i
                            lt = w3t[:, d, e * P:(e + 1) * P]
                            for qh in range(2):
                                nc.tensor.matmul(
                                    oyp[i][:, qh * 512:(qh + 1) * 512], lt,
                                    zt_sb[d][:, qh * 512:(qh + 1) * 512],
                                    start=(d == 0), stop=(d == ND - 1))
                else:
                    # final 2 tiles: per-tile qh-OUTER chains emitted as one
                    # uninterrupted 32-matmul block (a drain instruction
                    # between the chains invites the scheduler to hang a
                    # conservative cross-engine wait on the next chain start
                    # -- measured as a ~0.9us PE stall). The four stops land
                    # 1.7us apart; the drains, emitted after in stop order,
                    # pipeline behind them, so after the last matmul only
                    # one [P,512] half remains (ACT add -> act-queue DMA).
                    for i in range(2):
                        e = ebase + i
                        for qh in range(2):
                            for d in range(ND):
                                lt = w3t[:, d, e * P:(e + 1) * P]
                                nc.tensor.matmul(
                                    oyp[i][:, qh * 512:(qh + 1) * 512], lt,
                                    zt_sb[d][:, qh * 512:(qh + 1) * 512],
                                    start=(d == 0), stop=(d == ND - 1))
                    for i in range(2):
                        e = ebase + i
                        yt = ysb[e % 4]
                        for qh in range(2):
                            cs = slice(qh * 512, (qh + 1) * 512)
                            if qh == 0:
                                nc.vector.tensor_scalar_add(
                                    yt[:, cs], oyp[i][:, cs],
                                    biasc_sb[:, e:e + 1])
                                nc.sync.dma_start(
                                    yt_d[e * P:(e + 1) * P, cs], yt[:, cs])
                            else:
                                nc.scalar.activation(
                                    yt[:, cs], oyp[i][:, cs], Ident,
                                    bias=biasc_sb[:, e:e + 1])
                                nc.scalar.dma_start(
                                    yt_d[e * P:(e + 1) * P, cs], yt[:, cs])
                    continue
                for i in range(2):
                    e = ebase + i
                    yt = ysb[e % 4]
                    cs = slice(0, SQ)
                    if i == 0:
                        nc.vector.tensor_scalar_add(yt[:, cs],
                                                    oyp[i][:, cs],
                                                    biasc_sb[:, e:e + 1])
                        nc.sync.dma_start(yt_d[e * P:(e + 1) * P, cs],
                                          yt[:, cs])
                    else:
                        nc.scalar.activation(yt[:, cs], oyp[i][:, cs],
                                             Ident,
                                             bias=biasc_sb[:, e:e + 1])
                        nc.scalar.dma_start(yt_d[e * P:(e + 1) * P, cs],
                                            yt[:, cs])

    nc.compile()
    return nc


def _get_nc():
    if "nc" not in _CACHED:
        _CACHED["nc"] = _build_nc()
    return _CACHED["nc"]


def _fp8(a):
    return np.clip(a, -240.0, 240.0).astype(ml_dtypes.float8_e4m3fn)


def make_in_maps(x, w_qkv, w_proj, b_proj):
    wq = w_qkv[0:D]
    wk = w_qkv[D:2 * D]
    wv = w_qkv[2 * D:3 * D]
    w2 = wk.T @ wq                   # scoresT = Xk W2 Xq^T
    w3 = w_proj @ wv                 # y = (E X) W3^T / rowsum
    # W2'^T pair tiles [128, 4, 2, 1024]: [p, ee, i, d] = ALPHA*w2[d, .]
    w2tA = np.ascontiguousarray((ALPHA * w2).T)      # [e, d]
    w2t8 = _fp8(w2tA.reshape(4, 2, P, D).transpose(2, 0, 1, 3))
    w2t8 = np.ascontiguousarray(w2t8)
    w3t16 = np.ascontiguousarray(
        w3.T.astype(np.float16).reshape(8, P, D).transpose(1, 0, 2))
    biasc = np.ascontiguousarray(b_proj.reshape(8, P).T)
    in_maps = []
    for c in range(8):
        b, h = c // 2, c % 2
        own = x[b, h * SQ:(h + 1) * SQ]       # [1024, D] our queries
        other = x[b, (1 - h) * SQ:(2 - h) * SQ]
        xp = np.concatenate([own, other], axis=0)       # [2048, D] own-first
        xown8 = _fp8(own.T.reshape(4, 2, P, SQ).transpose(2, 0, 1, 3))
        xoth8 = _fp8(other.T.reshape(4, 2, P, SQ).transpose(2, 0, 1, 3))
        xrow16 = xp.astype(np.float16).reshape(16, P, D).transpose(1, 0, 2)
        in_maps.append({
            "xown8": np.ascontiguousarray(xown8),
            "xoth8": np.ascontiguousarray(xoth8),
            "w2t8": w2t8,
            "xrow": np.ascontiguousarray(xrow16),
            "w3t": w3t16,
            "biasc": biasc,
        })
    return in_maps


def gather_out(results):
    out = np.empty((4, S, D), dtype=np.float32)
    for c in range(8):
        b, h = c // 2, c % 2
        out[b, h * SQ:(h + 1) * SQ] = results[c]["yt"].astype(np.float32).T
    return out


def kernel(x, w_qkv, w_proj, b_proj):
    from concourse import bass_utils
    nc = _get_nc()
    in_maps = make_in_maps(np.asarray(x, dtype=np.float32),
                           np.asarray(w_qkv, dtype=np.float32),
                           np.asarray(w_proj, dtype=np.float32),
                           np.asarray(b_proj, dtype=np.float32))
    res = bass_utils.run_bass_kernel_spmd(nc, in_maps, list(range(8))).results
    return gather_out(res)

